# revision 24
# baseline (speedup 1.0000x reference)
"""MultiHeadAttention (B=4, S=2048, D=1024, H=16, causal + key mask) on 8 trn2 cores.

Sharding: Megatron-style tensor parallel over heads. Each core owns 2 heads:
column slices of Wq/Wk/Wv (D x 128), the matching row slice of Wp (128 x D).
Each core computes a partial output y_c = attn_c @ Wp_c; host sums the 8
partials and adds bp.

v4 (engine rebalance around the exp stream):
  - ScalarE is (almost) pure exp: q/k/v projection evictions moved to DVE
    tensor_scalar_add (bias fused, f32 psum -> bf16 sbuf in one op).
  - Normalize path unified for all batches: bf16 reciprocal row -> two bf16
    K=1 broadcast matmuls (213ns each, vs 859ns fp32) into PSUM; the
    normalize muls run on DVE reading the broadcast directly from PSUM.
    Kills the 41.6us of gpsimd DIRECT2D descriptor time and the tail stalls.
  - Output-projection evictions split DVE/ACT (3 of 8 per group on ACT) so
    both stay under the PE roofline.
  - xT is staged in DRAM pre-chunked [128, 16, 8, 512] so each chunk load is
    one contiguous 8KB line per partition (was 8x 1KB lines -> descriptor
    bound at ~11us/chunk).
  - Exp ACT table preloaded with a dummy activation during the proj phase.
  - v3 keeps: bf16 matmuls, row-tiled concurrent score pairs, multiplicative
    0/1 causal mask after exp, one-block score lookahead, PV ones-column
    denominator trick, batch b+1 projections interleaved into attention of
    batch b with lag-2 output projections.
"""

import numpy as np

P = 128
B, S, D, H = 4, 2048, 1024, 16
HD = D // H  # 64
NCORES = 8
HPC = H // NCORES  # 2 heads per core
BS = B * S  # 8192
NB = S // P  # 16 k-blocks per batch
NG = S // 512  # 4 q-groups per batch
NCHUNK = BS // 512  # 16 token chunks

_CACHE = {}


def _build_nc():
    import concourse.mybir as mybir
    from concourse import bacc
    from concourse.tile import TileContext
    from concourse.masks import make_identity
    from contextlib import ExitStack

    f32 = mybir.dt.float32
    bf16 = mybir.dt.bfloat16
    AF = mybir.ActivationFunctionType

    nc = bacc.Bacc("TRN2", target_bir_lowering=False, debug=False,
                   num_devices=NCORES)

    KD = D // P  # 8 contraction chunks
    # pre-chunked x^T: [p, chunk, o, m] = x^T[o*128+p, chunk*512+m]
    xT_d = nc.dram_tensor("xT", [P, NCHUNK, KD, 512], bf16,
                          kind="ExternalInput").ap()
    # weights pre-arranged [p, o, m] = W[o*128+p, m] (contiguous 2KB lines)
    wq_d = nc.dram_tensor("wq", [P, KD, P], bf16, kind="ExternalInput").ap()
    wk_d = nc.dram_tensor("wk", [P, KD, P], bf16, kind="ExternalInput").ap()
    wv_d = nc.dram_tensor("wv", [P, KD, P], bf16, kind="ExternalInput").ap()
    bq_d = nc.dram_tensor("bq", [P, 1], f32, kind="ExternalInput").ap()
    bk_d = nc.dram_tensor("bk", [P, 1], f32, kind="ExternalInput").ap()
    bv_d = nc.dram_tensor("bv", [P, 1], f32, kind="ExternalInput").ap()
    wp_d = nc.dram_tensor("wp", [P, D], bf16, kind="ExternalInput").ap()
    mb_d = nc.dram_tensor("maskb", [P, B * NB], f32, kind="ExternalInput").ap()
    cm_d = nc.dram_tensor("cmask", [P, 4, HPC, 512], bf16,
                          kind="ExternalInput").ap()
    yp_d = nc.dram_tensor("yp", [BS, D], bf16, kind="ExternalOutput").ap()

    with TileContext(nc) as tc:
        with ExitStack() as ctx:
            consts = ctx.enter_context(tc.tile_pool(name="consts", bufs=1))
            big = ctx.enter_context(tc.tile_pool(name="big", bufs=1))
            xpool = ctx.enter_context(tc.tile_pool(name="xpool", bufs=2))
            vtpool = ctx.enter_context(tc.tile_pool(name="vtpool", bufs=2))
            ptpool = ctx.enter_context(tc.tile_pool(name="ptpool", bufs=3))
            pvspool = ctx.enter_context(tc.tile_pool(name="pvs", bufs=2))
            npool = ctx.enter_context(tc.tile_pool(name="npool", bufs=2))
            ypool = ctx.enter_context(tc.tile_pool(name="ypool", bufs=4))
            psum = ctx.enter_context(
                tc.tile_pool(name="psum", bufs=2, space="PSUM"))
            sc2pool = ctx.enter_context(
                tc.tile_pool(name="sc2pool", bufs=2, space="PSUM"))
            pvpool = ctx.enter_context(
                tc.tile_pool(name="pvpool", bufs=2, space="PSUM"))

            # ---- constants (critical path first: wq/bq gate proj chunk 0)
            wq_sb = consts.tile([P, KD, P], bf16, tag="wq")
            wk_sb = consts.tile([P, KD, P], bf16, tag="wk")
            wv_sb = consts.tile([P, KD, P], bf16, tag="wv")
            bq_sb = consts.tile([P, 1], f32, tag="bq")
            bk_sb = consts.tile([P, 1], f32, tag="bk")
            bv_sb = consts.tile([P, 1], f32, tag="bv")
            nc.sync.dma_start(wq_sb[:], wq_d)
            nc.sync.dma_start(bq_sb[:], bq_d)
            nc.sync.dma_start(wk_sb[:], wk_d)
            nc.sync.dma_start(bk_sb[:], bk_d)
            nc.sync.dma_start(wv_sb[:], wv_d)
            nc.sync.dma_start(bv_sb[:], bv_d)
            wp_sb = consts.tile([P, D], bf16, tag="wp")
            nc.sync.dma_start(wp_sb[:], wp_d)
            # rows 64..127 of Wp re-homed at partitions 0..63 for the final
            # group's split output projection (lhsT/rhs base partitions must
            # match)
            wp2_sb = consts.tile([HD, D], bf16, tag="wp2")
            nc.sync.dma_start(wp2_sb[:], wp_d[HD:P, :])
            mb_sb = consts.tile([P, B * NB], f32, tag="mb")
            nc.sync.dma_start(mb_sb[:], mb_d)
            # multiplicative causal masks, [p, j, head, q'] 0/1 bf16
            cm_sb = consts.tile([P, 4, HPC, 512], bf16, tag="cm")
            nc.sync.dma_start(cm_sb[:], cm_d)
            ident = consts.tile([P, P], bf16, tag="ident")
            make_identity(nc, ident[:])
            # ones row on partition 64 (same partition as the PV denominator
            # row) -- bf16 lhsT of the reciprocal-broadcast matmul
            ones64 = consts.tile([P, HD], bf16, tag="ones64")
            nc.vector.memset(ones64[HD:HD + 1, :], 1.0)
            # scratch for the exp ACT-table preload
            warm = consts.tile([P, 1], f32, tag="warm")
            nc.scalar.activation(warm[:], bq_sb[:], AF.Exp)

            # ---- persistent activations (all bf16) ----
            qt_sb = big.tile([P, B, S], bf16, tag="qt")  # Q^T
            kt_sb = big.tile([P, B, S], bf16, tag="kt")  # K^T
            # V in [s, hd] layout + ones col: [p=s%128, b, sblock, h, 65]
            v_sb = big.tile([P, B, NB, HPC, HD + 1], bf16, tag="v")
            at_sb = big.tile([P, B, S], bf16, tag="at")  # attn^T (normalized)
            nc.vector.memset(v_sb[:, :, :, :, HD], 1.0)

            # ---- projections for one 512-row chunk of x ----
            def proj_chunk(c):
                b, sc = divmod(c, NG)
                xt = xpool.tile([P, KD, 512], bf16, tag="xt")
                # gpsimd DMA queue: the sync queue carries the y writes whose
                # in-queue semaphore waits would head-of-line block this load
                nc.gpsimd.dma_start(xt[:], xT_d[:, c, :, :])
                ssl = slice(sc * 512, (sc + 1) * 512)
                for which in range(3):
                    w_sb = (wq_sb, wk_sb, wv_sb)[which]
                    ps = psum.tile([P, 512], f32, tag="ps")
                    for o in range(KD):
                        nc.tensor.matmul(
                            ps[:], lhsT=w_sb[:, o, :], rhs=xt[:, o, :],
                            start=(o == 0), stop=(o == KD - 1))
                    if which == 0:
                        nc.vector.tensor_scalar_add(qt_sb[:, b, ssl], ps[:],
                                                    bq_sb[:])
                    elif which == 1:
                        nc.vector.tensor_scalar_add(kt_sb[:, b, ssl], ps[:],
                                                    bk_sb[:])
                    else:
                        vt = vtpool.tile([P, 512], bf16, tag="vt")
                        nc.vector.tensor_scalar_add(vt[:], ps[:], bv_sb[:])
                        for t in range(4):
                            # shares the "ps" slots (pools size per tag)
                            trp = psum.tile([P, P], bf16, tag="ps")
                            nc.tensor.transpose(
                                trp[:], vt[:, t * P:(t + 1) * P], ident[:])
                            sb_i = sc * 4 + t
                            nc.vector.tensor_copy(
                                v_sb[:, b, sb_i, 0, 0:HD], trp[:, 0:HD])
                            nc.vector.tensor_copy(
                                v_sb[:, b, sb_i, 1, 0:HD],
                                trp[:, HD:2 * HD])

            # ---- output projection for one (b, g) q-group ----
            def outproj(b, g):
                for qc in range(4):
                    q0 = g * 512 + qc * P
                    r0 = b * S + q0
                    y_sb = ypool.tile([P, 2, 512], bf16, tag="y",
                                      name=f"y_{b}_{g}_{qc}")
                    for half in range(2):
                        yp_ps = psum.tile([P, 512], f32, tag="ps",
                                          name=f"yps_{b}_{g}_{qc}_{half}")
                        nc.tensor.matmul(
                            yp_ps[:],
                            lhsT=at_sb[:, b, q0:q0 + P],
                            rhs=wp_sb[:, half * 512:(half + 1) * 512],
                            start=True, stop=True)
                        if half == 1 and qc != 0:
                            # 3 of 8 evictions per group ride on ScalarE to
                            # balance ACT/DVE under the PE roofline
                            nc.scalar.activation(y_sb[:, half, :], yp_ps[:],
                                                 AF.Identity)
                        else:
                            nc.vector.tensor_copy(y_sb[:, half, :], yp_ps[:])
                    nc.sync.dma_start(
                        yp_d[r0:r0 + P, :],
                        y_sb[:].rearrange("p a n -> p (a n)"))

            # ---- attention for one (b, g) q-group ----
            def attn_group(b, g, pending, split_out=False):
                gsl = slice(g * 512, (g + 1) * 512)
                nkb = 4 * (g + 1)
                # diagonal (masked) k-blocks FIRST: their longer
                # exp->mask->PV chains overlap the pipeline fill, and the
                # group drains through clean full blocks at the interlock
                # rate
                order = list(range(4 * g, nkb)) + list(range(4 * g))
                pvs = [pvpool.tile([P, 512], f32, tag="pv",
                                   name=f"pv_{b}_{g}_{h}")
                       for h in range(HPC)]

                def scores(kb):
                    j = kb - 4 * g
                    # diagonal blocks: q < 128*j is fully masked
                    qo = 128 * max(j, 0)
                    sc2 = sc2pool.tile([P, HPC, 512], f32, tag="sc2",
                                       name=f"sc2_{b}_{g}_{kb}")
                    for h in range(HPC):
                        hsl = slice(h * HD, (h + 1) * HD)
                        nc.tensor.matmul(
                            sc2[:, h, qo:512],
                            lhsT=kt_sb[hsl, b, kb * P:(kb + 1) * P],
                            rhs=qt_sb[hsl, b, g * 512 + qo:(g + 1) * 512],
                            start=True, stop=True)
                    return sc2, kb, j, qo

                cur = scores(order[0])
                for i, kb in enumerate(order):
                    nxt = scores(order[i + 1]) if i + 1 < nkb else None
                    sc2, _, j, qo = cur
                    col = b * NB + kb
                    pt = ptpool.tile([P, HPC, 512], bf16, tag="pt")
                    if qo == 0:
                        nc.scalar.activation(pt[:], sc2[:], AF.Exp,
                                             bias=mb_sb[:, col:col + 1])
                        if 0 <= j:  # multiplicative 0/1 causal mask
                            nc.vector.tensor_mul(pt[:], pt[:],
                                                 cm_sb[:, j, :, :])
                    else:
                        nc.scalar.activation(pt[:, :, qo:512],
                                             sc2[:, :, qo:512], AF.Exp,
                                             bias=mb_sb[:, col:col + 1])
                        # narrow diagonal masks ride the otherwise-idle
                        # GpSimd; wide ones stay on DVE
                        eng = nc.gpsimd if j >= 2 else nc.vector
                        eng.tensor_mul(pt[:, :, qo:512],
                                       pt[:, :, qo:512],
                                       cm_sb[:, j, :, qo:512])
                    for h in range(HPC):
                        nc.tensor.matmul(
                            pvs[h][0:HD + 1, qo:512],
                            lhsT=v_sb[:, b, kb, h, :],
                            rhs=pt[:, h, qo:512],
                            start=(i == 0), stop=(i == nkb - 1))
                    cur = nxt
                # evict PV psums immediately so the banks recycle without
                # waiting on the normalize chain
                pvs_sb = pvspool.tile([P, HPC, 512], f32, tag="pvs")
                for h in range(HPC):
                    nc.vector.tensor_copy(pvs_sb[0:HD + 1, h, :],
                                          pvs[h][0:HD + 1, :])
                if not split_out:
                    pending.append((b, g))
                # ---- normalize: 1/denom (row 64, bf16) -> K=1 bf16 matmul
                # broadcast into PSUM -> DVE muls reading PSUM ----
                # reciprocal_approx_fast misbehaves on single-partition
                # slices -- run it over the full tile (unused rows discarded)
                rcp = npool.tile([P, HPC, 512], f32, tag="rcp")
                nc.vector.reciprocal_approx_fast(rcp[:], pvs_sb[:])
                # bf16 copy of the denominator-reciprocal row: keeps the
                # broadcast matmul at 1 cyc/row (fp32 rhs would be 4x)
                rcpb = npool.tile([P, HPC, 512], bf16, tag="rcpb")
                nc.vector.tensor_copy(rcpb[HD:HD + 1, :, :],
                                      rcp[HD:HD + 1, :, :])
                bc = [pvpool.tile([P, 512], f32, tag="pv",
                                  name=f"bc_{b}_{g}_{h}")
                      for h in range(HPC)]
                for h in range(HPC):
                    nc.tensor.matmul(
                        bc[h][0:HD, :], lhsT=ones64[HD:HD + 1, :],
                        rhs=rcpb[HD:HD + 1, h, :], start=True, stop=True)
                tmp = npool.tile([HD, 512], bf16, tag="tmp")
                if split_out:
                    # final group: keep both halves at partitions 0..63 and
                    # feed the split output projection directly -- no at_sb
                    # partition-shift DMA in the tail chain
                    a0 = npool.tile([HD, 512], bf16, tag="a0")
                    nc.vector.tensor_mul(a0[:], pvs_sb[0:HD, 0, :],
                                         bc[0][0:HD, :])
                    nc.vector.tensor_mul(tmp[:], pvs_sb[0:HD, 1, :],
                                         bc[1][0:HD, :])
                    return a0, tmp
                nc.vector.tensor_mul(at_sb[0:HD, b, gsl],
                                     pvs_sb[0:HD, 0, :], bc[0][0:HD, :])
                nc.vector.tensor_mul(tmp[:], pvs_sb[0:HD, 1, :],
                                     bc[1][0:HD, :])
                nc.gpsimd.dma_start(at_sb[HD:2 * HD, b, gsl], tmp[:])
                return None

            # ---- split output projection for the final group: two k=64
            # accumulating matmuls per psum, lhsT halves at partitions 0-63
            def outproj_split(b, g, a0, a1):
                for qc in range(4):
                    q0 = g * 512 + qc * P
                    r0 = b * S + q0
                    qsl = slice(qc * P, (qc + 1) * P)
                    y_sb = ypool.tile([P, 2, 512], bf16, tag="y",
                                      name=f"ys_{b}_{g}_{qc}")
                    for half in range(2):
                        nsl = slice(half * 512, (half + 1) * 512)
                        yp_ps = psum.tile([P, 512], f32, tag="ps",
                                          name=f"yss_{b}_{g}_{qc}_{half}")
                        nc.tensor.matmul(
                            yp_ps[:], lhsT=a0[:, qsl],
                            rhs=wp_sb[0:HD, nsl], start=True, stop=False)
                        nc.tensor.matmul(
                            yp_ps[:], lhsT=a1[:, qsl],
                            rhs=wp2_sb[:, nsl], start=False, stop=True)
                        nc.vector.tensor_copy(y_sb[:, half, :], yp_ps[:])
                    nc.sync.dma_start(
                        yp_d[r0:r0 + P, :],
                        y_sb[:].rearrange("p a n -> p (a n)"))

            # ---- schedule: uniform pipeline. proj chunks run 2 groups ahead
            # of the attention that consumes them, so batch-0 attention (and
            # its exp stream) starts ~8us in instead of after a dead 22us
            # proj-only phase. Output projections are deferred into the
            # ACT-bound batch-3 window via the drain table. ----
            DRAIN = [[0, 0, 0, 1], [0, 1, 1, 1], [1, 1, 1, 1], [2, 2, 2, 9]]
            proj_chunk(0)
            proj_chunk(1)
            pending = []
            for b in range(B):
                for g in range(NG):
                    last = (b == B - 1 and g == NG - 1)
                    ret = attn_group(b, g, pending, split_out=last)
                    for _ in range(DRAIN[b][g]):
                        if pending:
                            outproj(*pending.pop(0))
                    if last:
                        outproj_split(b, g, *ret)
                    c = NG * b + g + 2
                    if c < NCHUNK:
                        proj_chunk(c)
            for pg in pending:
                outproj(*pg)

    nc.compile()
    return nc


def _get_nc():
    if "nc" not in _CACHE:
        _CACHE["nc"] = _build_nc()
    return _CACHE["nc"]


def make_in_maps(x, attention_mask, Wq, bq, Wk, bk, Wv, bv, Wp, bp):
    """Host-side sharding: build the 8 per-core device input maps."""
    import ml_dtypes
    bf16 = ml_dtypes.bfloat16
    KD8 = D // P
    x = np.asarray(x, dtype=np.float32)
    scale = np.float32(1.0 / np.sqrt(HD))
    xT = x.reshape(BS, D).T.astype(bf16)  # [D, BS]
    # pre-chunked layout: [p, chunk, o, m] = xT[o*128+p, chunk*512+m]
    xTc = np.ascontiguousarray(
        xT.reshape(D // P, P, NCHUNK, 512).transpose(1, 2, 0, 3))
    mb = (np.asarray(attention_mask).astype(np.float32) - 1.0) * np.float32(1e9)
    mb = np.ascontiguousarray(
        mb.reshape(B, NB, P).transpose(2, 0, 1).reshape(P, B * NB))
    # multiplicative causal masks: 1 where 128*j + p <= q', else 0;
    # duplicated for the two heads: [128, 4, 2, 512]
    pp = np.arange(P)[:, None]
    ff = np.arange(512)[None, :]
    cm = np.stack(
        [np.where(P * j + pp <= ff, 1.0, 0.0).astype(bf16)
         for j in range(4)], axis=1)  # [128, 4, 512]
    cm = np.ascontiguousarray(
        np.broadcast_to(cm[:, :, None, :], (P, 4, HPC, 512)))

    Wq = (np.asarray(Wq, np.float32) * scale).astype(bf16)
    bq = np.asarray(bq, np.float32) * scale
    Wk = np.asarray(Wk, np.float32).astype(bf16)
    bk = np.asarray(bk, np.float32)
    Wv = np.asarray(Wv, np.float32).astype(bf16)
    bv = np.asarray(bv, np.float32)
    Wp = np.asarray(Wp, np.float32).astype(bf16)

    def wrearr(w, cs):
        # [1024, 128] core slice -> [p, o, m] = W[o*128+p, m]
        return np.ascontiguousarray(
            w[:, cs].reshape(KD8, P, P).transpose(1, 0, 2))

    in_maps = []
    for c in range(NCORES):
        cs = slice(c * P, (c + 1) * P)
        in_maps.append({
            "xT": xTc,
            "wq": wrearr(Wq, cs),
            "wk": wrearr(Wk, cs),
            "wv": wrearr(Wv, cs),
            "bq": np.ascontiguousarray(bq[cs].reshape(P, 1)),
            "bk": np.ascontiguousarray(bk[cs].reshape(P, 1)),
            "bv": np.ascontiguousarray(bv[cs].reshape(P, 1)),
            "wp": np.ascontiguousarray(Wp[cs, :]),
            "maskb": mb,
            "cmask": cm,
        })
    return in_maps


def run(inputs, trace=False, tmpdir=None):
    """Compile (cached) + run on 8 cores. Returns (output, BassKernelResults)."""
    from concourse import bass_utils
    nc = _get_nc()
    in_maps = make_in_maps(**inputs)
    kwargs = {}
    if trace:
        kwargs = dict(trace=True, tmpdir=tmpdir)
    res = bass_utils.run_bass_kernel_spmd(
        nc, in_maps, core_ids=list(range(NCORES)), **kwargs)
    acc = np.zeros((BS, D), dtype=np.float32)
    for r in res.results:
        acc += r["yp"].astype(np.float32)
    out = acc + np.asarray(inputs["bp"], np.float32)[None, :]
    return out.reshape(B, S, D), res


def kernel(**inputs) -> np.ndarray:
    out, _ = run(inputs, trace=False)
    return out


# revision 27
# speedup vs baseline: 1.0580x; 1.0580x over previous
"""MultiHeadAttention (B=4, S=2048, D=1024, H=16, causal + key mask) on 8 trn2 cores.

Sharding: Megatron-style tensor parallel over heads. Each core owns 2 heads:
column slices of Wq/Wk/Wv (D x 128), the matching row slice of Wp (128 x D).
Each core computes a partial output y_c = attn_c @ Wp_c; host sums the 8
partials and adds bp.

v4 (engine rebalance around the exp stream):
  - ScalarE is (almost) pure exp: q/k/v projection evictions moved to DVE
    tensor_scalar_add (bias fused, f32 psum -> bf16 sbuf in one op).
  - Normalize path unified for all batches: bf16 reciprocal row -> two bf16
    K=1 broadcast matmuls (213ns each, vs 859ns fp32) into PSUM; the
    normalize muls run on DVE reading the broadcast directly from PSUM.
    Kills the 41.6us of gpsimd DIRECT2D descriptor time and the tail stalls.
  - Output-projection evictions split DVE/ACT (3 of 8 per group on ACT) so
    both stay under the PE roofline.
  - xT is staged in DRAM pre-chunked [128, 16, 8, 512] so each chunk load is
    one contiguous 8KB line per partition (was 8x 1KB lines -> descriptor
    bound at ~11us/chunk).
  - Exp ACT table preloaded with a dummy activation during the proj phase.
  - v3 keeps: bf16 matmuls, row-tiled concurrent score pairs, multiplicative
    0/1 causal mask after exp, one-block score lookahead, PV ones-column
    denominator trick, batch b+1 projections interleaved into attention of
    batch b with lag-2 output projections.
"""

import numpy as np

P = 128
B, S, D, H = 4, 2048, 1024, 16
HD = D // H  # 64
NCORES = 8
HPC = H // NCORES  # 2 heads per core
BS = B * S  # 8192
NB = S // P  # 16 k-blocks per batch
NG = S // 512  # 4 q-groups per batch
NCHUNK = BS // 512  # 16 token chunks

_CACHE = {}


def _build_nc():
    import concourse.mybir as mybir
    from concourse import bacc
    from concourse.tile import TileContext
    from concourse.masks import make_identity
    from contextlib import ExitStack

    f32 = mybir.dt.float32
    bf16 = mybir.dt.bfloat16
    AF = mybir.ActivationFunctionType

    nc = bacc.Bacc("TRN2", target_bir_lowering=False, debug=False,
                   num_devices=NCORES)

    KD = D // P  # 8 contraction chunks
    # pre-chunked x^T: [p, chunk, o, m] = x^T[o*128+p, chunk*512+m]
    xT_d = nc.dram_tensor("xT", [P, NCHUNK, KD, 512], bf16,
                          kind="ExternalInput").ap()
    # weights pre-arranged [p, o, m] = W[o*128+p, m] (contiguous 2KB lines)
    wq_d = nc.dram_tensor("wq", [P, KD, P], bf16, kind="ExternalInput").ap()
    wk_d = nc.dram_tensor("wk", [P, KD, P], bf16, kind="ExternalInput").ap()
    wv_d = nc.dram_tensor("wv", [P, KD, P], bf16, kind="ExternalInput").ap()
    bq_d = nc.dram_tensor("bq", [P, 1], f32, kind="ExternalInput").ap()
    bk_d = nc.dram_tensor("bk", [P, 1], f32, kind="ExternalInput").ap()
    bv_d = nc.dram_tensor("bv", [P, 1], f32, kind="ExternalInput").ap()
    wp_d = nc.dram_tensor("wp", [P, D], bf16, kind="ExternalInput").ap()
    mb_d = nc.dram_tensor("maskb", [P, B * NB], f32, kind="ExternalInput").ap()
    cm_d = nc.dram_tensor("cmask", [P, 4, HPC, 512], bf16,
                          kind="ExternalInput").ap()
    yp_d = nc.dram_tensor("yp", [BS, D], bf16, kind="ExternalOutput").ap()

    with TileContext(nc) as tc:
        with ExitStack() as ctx:
            consts = ctx.enter_context(tc.tile_pool(name="consts", bufs=1))
            big = ctx.enter_context(tc.tile_pool(name="big", bufs=1))
            xpool = ctx.enter_context(tc.tile_pool(name="xpool", bufs=2))
            vtpool = ctx.enter_context(tc.tile_pool(name="vtpool", bufs=2))
            ptpool = ctx.enter_context(tc.tile_pool(name="ptpool", bufs=3))
            pvspool = ctx.enter_context(tc.tile_pool(name="pvs", bufs=2))
            npool = ctx.enter_context(tc.tile_pool(name="npool", bufs=2))
            ypool = ctx.enter_context(tc.tile_pool(name="ypool", bufs=4))
            psum = ctx.enter_context(
                tc.tile_pool(name="psum", bufs=2, space="PSUM"))
            sc2pool = ctx.enter_context(
                tc.tile_pool(name="sc2pool", bufs=2, space="PSUM"))
            pvpool = ctx.enter_context(
                tc.tile_pool(name="pvpool", bufs=2, space="PSUM"))

            # ---- constants (critical path first: wq/bq gate proj chunk 0)
            wq_sb = consts.tile([P, KD, P], bf16, tag="wq")
            wk_sb = consts.tile([P, KD, P], bf16, tag="wk")
            wv_sb = consts.tile([P, KD, P], bf16, tag="wv")
            bq_sb = consts.tile([P, 1], f32, tag="bq")
            bk_sb = consts.tile([P, 1], f32, tag="bk")
            bv_sb = consts.tile([P, 1], f32, tag="bv")
            nc.sync.dma_start(wq_sb[:], wq_d)
            nc.sync.dma_start(bq_sb[:], bq_d)
            nc.sync.dma_start(wk_sb[:], wk_d)
            nc.sync.dma_start(bk_sb[:], bk_d)
            nc.sync.dma_start(wv_sb[:], wv_d)
            nc.sync.dma_start(bv_sb[:], bv_d)
            wp_sb = consts.tile([P, D], bf16, tag="wp")
            nc.sync.dma_start(wp_sb[:], wp_d)
            # rows 64..127 of Wp re-homed at partitions 0..63 for the final
            # group's split output projection (lhsT/rhs base partitions must
            # match)
            wp2_sb = consts.tile([HD, D], bf16, tag="wp2")
            nc.sync.dma_start(wp2_sb[:], wp_d[HD:P, :])
            mb_sb = consts.tile([P, B * NB], f32, tag="mb")
            nc.sync.dma_start(mb_sb[:], mb_d)
            # multiplicative causal masks, [p, j, head, q'] 0/1 bf16
            cm_sb = consts.tile([P, 4, HPC, 512], bf16, tag="cm")
            nc.sync.dma_start(cm_sb[:], cm_d)
            ident = consts.tile([P, P], bf16, tag="ident")
            make_identity(nc, ident[:])
            # ones row on partition 64 (same partition as the PV denominator
            # row) -- bf16 lhsT of the reciprocal-broadcast matmul
            ones64 = consts.tile([P, HD], bf16, tag="ones64")
            nc.vector.memset(ones64[HD:HD + 1, :], 1.0)
            # scratch for the exp ACT-table preload
            warm = consts.tile([P, 1], f32, tag="warm")
            nc.scalar.activation(warm[:], bq_sb[:], AF.Exp)

            # ---- persistent activations (all bf16) ----
            qt_sb = big.tile([P, B, S], bf16, tag="qt")  # Q^T
            kt_sb = big.tile([P, B, S], bf16, tag="kt")  # K^T
            # V in [s, hd] layout + ones col: [p=s%128, b, sblock, h, 65]
            v_sb = big.tile([P, B, NB, HPC, HD + 1], bf16, tag="v")
            at_sb = big.tile([P, B, S], bf16, tag="at")  # attn^T (normalized)
            nc.vector.memset(v_sb[:, :, :, :, HD], 1.0)

            # ---- projections for one 512-row chunk of x ----
            def proj_chunk(c):
                b, sc = divmod(c, NG)
                xt = xpool.tile([P, KD, 512], bf16, tag="xt")
                # gpsimd DMA queue: the sync queue carries the y writes whose
                # in-queue semaphore waits would head-of-line block this load
                nc.gpsimd.dma_start(xt[:], xT_d[:, c, :, :])
                ssl = slice(sc * 512, (sc + 1) * 512)
                for which in range(3):
                    w_sb = (wq_sb, wk_sb, wv_sb)[which]
                    ps = psum.tile([P, 512], f32, tag="ps")
                    for o in range(KD):
                        nc.tensor.matmul(
                            ps[:], lhsT=w_sb[:, o, :], rhs=xt[:, o, :],
                            start=(o == 0), stop=(o == KD - 1))
                    if which == 0:
                        nc.vector.tensor_scalar_add(qt_sb[:, b, ssl], ps[:],
                                                    bq_sb[:])
                    elif which == 1:
                        nc.vector.tensor_scalar_add(kt_sb[:, b, ssl], ps[:],
                                                    bk_sb[:])
                    else:
                        vt = vtpool.tile([P, 512], bf16, tag="vt")
                        nc.vector.tensor_scalar_add(vt[:], ps[:], bv_sb[:])
                        for t in range(4):
                            # shares the "ps" slots (pools size per tag)
                            trp = psum.tile([P, P], bf16, tag="ps")
                            nc.tensor.transpose(
                                trp[:], vt[:, t * P:(t + 1) * P], ident[:])
                            sb_i = sc * 4 + t
                            nc.vector.tensor_copy(
                                v_sb[:, b, sb_i, 0, 0:HD], trp[:, 0:HD])
                            nc.vector.tensor_copy(
                                v_sb[:, b, sb_i, 1, 0:HD],
                                trp[:, HD:2 * HD])

            # ---- output projection for one (b, g) q-group ----
            def outproj(b, g):
                for qc in range(4):
                    q0 = g * 512 + qc * P
                    r0 = b * S + q0
                    y_sb = ypool.tile([P, 2, 512], bf16, tag="y",
                                      name=f"y_{b}_{g}_{qc}")
                    for half in range(2):
                        yp_ps = psum.tile([P, 512], f32, tag="ps",
                                          name=f"yps_{b}_{g}_{qc}_{half}")
                        nc.tensor.matmul(
                            yp_ps[:],
                            lhsT=at_sb[:, b, q0:q0 + P],
                            rhs=wp_sb[:, half * 512:(half + 1) * 512],
                            start=True, stop=True)
                        if half == 1 and qc != 0:
                            # 3 of 8 evictions per group ride on ScalarE to
                            # balance ACT/DVE under the PE roofline
                            nc.scalar.activation(y_sb[:, half, :], yp_ps[:],
                                                 AF.Identity)
                        else:
                            nc.vector.tensor_copy(y_sb[:, half, :], yp_ps[:])
                    nc.sync.dma_start(
                        yp_d[r0:r0 + P, :],
                        y_sb[:].rearrange("p a n -> p (a n)"))

            # ---- attention for one (b, g) q-group ----
            def attn_group(b, g, pending, split_out=False):
                gsl = slice(g * 512, (g + 1) * 512)
                nkb = 4 * (g + 1)
                order = list(range(nkb))
                pvs = [pvpool.tile([P, 512], f32, tag="pv",
                                   name=f"pv_{b}_{g}_{h}")
                       for h in range(HPC)]

                def scores(kb):
                    j = kb - 4 * g
                    # diagonal blocks: q < 128*j is fully masked
                    qo = 128 * max(j, 0)
                    sc2 = sc2pool.tile([P, HPC, 512], f32, tag="sc2",
                                       name=f"sc2_{b}_{g}_{kb}")
                    for h in range(HPC):
                        hsl = slice(h * HD, (h + 1) * HD)
                        nc.tensor.matmul(
                            sc2[:, h, qo:512],
                            lhsT=kt_sb[hsl, b, kb * P:(kb + 1) * P],
                            rhs=qt_sb[hsl, b, g * 512 + qo:(g + 1) * 512],
                            start=True, stop=True)
                    return sc2, kb, j, qo

                cur = scores(order[0])
                for i, kb in enumerate(order):
                    nxt = scores(order[i + 1]) if i + 1 < nkb else None
                    sc2, _, j, qo = cur
                    col = b * NB + kb
                    pt = ptpool.tile([P, HPC, 512], bf16, tag="pt")
                    if qo == 0:
                        nc.scalar.activation(pt[:], sc2[:], AF.Exp,
                                             bias=mb_sb[:, col:col + 1])
                    else:
                        nc.scalar.activation(pt[:, :, qo:512],
                                             sc2[:, :, qo:512], AF.Exp,
                                             bias=mb_sb[:, col:col + 1])
                    if j >= 0:
                        # causal mask: only the 128-col strip [qo, qo+128)
                        # is triangular -- mask just that strip so the wide
                        # clean part of PV never waits on the mask-mul
                        nc.vector.tensor_mul(pt[:, :, qo:qo + P],
                                             pt[:, :, qo:qo + P],
                                             cm_sb[:, j, :, qo:qo + P])
                    for h in range(HPC):
                        if j >= 0 and qo + P < 512:
                            # clean columns: chain is exp -> PV directly
                            nc.tensor.matmul(
                                pvs[h][0:HD + 1, qo + P:512],
                                lhsT=v_sb[:, b, kb, h, :],
                                rhs=pt[:, h, qo + P:512],
                                start=(i == 0), stop=False)
                            nc.tensor.matmul(
                                pvs[h][0:HD + 1, qo:qo + P],
                                lhsT=v_sb[:, b, kb, h, :],
                                rhs=pt[:, h, qo:qo + P],
                                start=False, stop=(i == nkb - 1))
                        else:
                            nc.tensor.matmul(
                                pvs[h][0:HD + 1, qo:512],
                                lhsT=v_sb[:, b, kb, h, :],
                                rhs=pt[:, h, qo:512],
                                start=(i == 0), stop=(i == nkb - 1))
                    cur = nxt
                # evict PV psums immediately so the banks recycle without
                # waiting on the normalize chain
                pvs_sb = pvspool.tile([P, HPC, 512], f32, tag="pvs")
                for h in range(HPC):
                    nc.vector.tensor_copy(pvs_sb[0:HD + 1, h, :],
                                          pvs[h][0:HD + 1, :])
                if not split_out:
                    pending.append((b, g))
                # ---- normalize: 1/denom (row 64, bf16) -> K=1 bf16 matmul
                # broadcast into PSUM -> DVE muls reading PSUM ----
                # reciprocal_approx_fast misbehaves on single-partition
                # slices -- run it over the full tile (unused rows discarded)
                rcp = npool.tile([P, HPC, 512], f32, tag="rcp")
                nc.vector.reciprocal_approx_fast(rcp[:], pvs_sb[:])
                # bf16 copy of the denominator-reciprocal row: keeps the
                # broadcast matmul at 1 cyc/row (fp32 rhs would be 4x)
                rcpb = npool.tile([P, HPC, 512], bf16, tag="rcpb")
                nc.vector.tensor_copy(rcpb[HD:HD + 1, :, :],
                                      rcp[HD:HD + 1, :, :])
                bc = [pvpool.tile([P, 512], f32, tag="pv",
                                  name=f"bc_{b}_{g}_{h}")
                      for h in range(HPC)]
                for h in range(HPC):
                    nc.tensor.matmul(
                        bc[h][0:HD, :], lhsT=ones64[HD:HD + 1, :],
                        rhs=rcpb[HD:HD + 1, h, :], start=True, stop=True)
                tmp = npool.tile([HD, 512], bf16, tag="tmp")
                if split_out:
                    # final group: keep both halves at partitions 0..63 and
                    # feed the split output projection directly -- no at_sb
                    # partition-shift DMA in the tail chain
                    a0 = npool.tile([HD, 512], bf16, tag="a0")
                    nc.vector.tensor_mul(a0[:], pvs_sb[0:HD, 0, :],
                                         bc[0][0:HD, :])
                    nc.vector.tensor_mul(tmp[:], pvs_sb[0:HD, 1, :],
                                         bc[1][0:HD, :])
                    return a0, tmp
                nc.vector.tensor_mul(at_sb[0:HD, b, gsl],
                                     pvs_sb[0:HD, 0, :], bc[0][0:HD, :])
                nc.vector.tensor_mul(tmp[:], pvs_sb[0:HD, 1, :],
                                     bc[1][0:HD, :])
                nc.gpsimd.dma_start(at_sb[HD:2 * HD, b, gsl], tmp[:])
                return None

            # ---- split output projection for the final group: two k=64
            # accumulating matmuls per psum, lhsT halves at partitions 0-63
            def outproj_split(b, g, a0, a1):
                for qc in range(4):
                    q0 = g * 512 + qc * P
                    r0 = b * S + q0
                    qsl = slice(qc * P, (qc + 1) * P)
                    y_sb = ypool.tile([P, 2, 512], bf16, tag="y",
                                      name=f"ys_{b}_{g}_{qc}")
                    for half in range(2):
                        nsl = slice(half * 512, (half + 1) * 512)
                        yp_ps = psum.tile([P, 512], f32, tag="ps",
                                          name=f"yss_{b}_{g}_{qc}_{half}")
                        nc.tensor.matmul(
                            yp_ps[:], lhsT=a0[:, qsl],
                            rhs=wp_sb[0:HD, nsl], start=True, stop=False)
                        nc.tensor.matmul(
                            yp_ps[:], lhsT=a1[:, qsl],
                            rhs=wp2_sb[:, nsl], start=False, stop=True)
                        nc.vector.tensor_copy(y_sb[:, half, :], yp_ps[:])
                    nc.sync.dma_start(
                        yp_d[r0:r0 + P, :],
                        y_sb[:].rearrange("p a n -> p (a n)"))

            # ---- schedule: uniform pipeline. proj chunks run 2 groups ahead
            # of the attention that consumes them, so batch-0 attention (and
            # its exp stream) starts ~8us in instead of after a dead 22us
            # proj-only phase. Output projections are deferred into the
            # ACT-bound batch-3 window via the drain table. ----
            DRAIN = [[0, 0, 1, 1], [0, 1, 1, 1], [0, 0, 1, 1], [2, 2, 2, 9]]
            proj_chunk(0)
            pending = []
            for b in range(B):
                for g in range(NG):
                    last = (b == B - 1 and g == NG - 1)
                    ret = attn_group(b, g, pending, split_out=last)
                    for _ in range(DRAIN[b][g]):
                        if pending:
                            outproj(*pending.pop(0))
                    if last:
                        outproj_split(b, g, *ret)
                    # 1-ahead proj cadence: keeps chunks 13-15 as batch-3
                    # PE filler while staying a full group ahead of use
                    c = NG * b + g + 1
                    if c < NCHUNK:
                        proj_chunk(c)
            for pg in pending:
                outproj(*pg)

    nc.compile()
    return nc


def _get_nc():
    if "nc" not in _CACHE:
        _CACHE["nc"] = _build_nc()
    return _CACHE["nc"]


def make_in_maps(x, attention_mask, Wq, bq, Wk, bk, Wv, bv, Wp, bp):
    """Host-side sharding: build the 8 per-core device input maps."""
    import ml_dtypes
    bf16 = ml_dtypes.bfloat16
    KD8 = D // P
    x = np.asarray(x, dtype=np.float32)
    scale = np.float32(1.0 / np.sqrt(HD))
    xT = x.reshape(BS, D).T.astype(bf16)  # [D, BS]
    # pre-chunked layout: [p, chunk, o, m] = xT[o*128+p, chunk*512+m]
    xTc = np.ascontiguousarray(
        xT.reshape(D // P, P, NCHUNK, 512).transpose(1, 2, 0, 3))
    mb = (np.asarray(attention_mask).astype(np.float32) - 1.0) * np.float32(1e9)
    mb = np.ascontiguousarray(
        mb.reshape(B, NB, P).transpose(2, 0, 1).reshape(P, B * NB))
    # multiplicative causal masks: 1 where 128*j + p <= q', else 0;
    # duplicated for the two heads: [128, 4, 2, 512]
    pp = np.arange(P)[:, None]
    ff = np.arange(512)[None, :]
    cm = np.stack(
        [np.where(P * j + pp <= ff, 1.0, 0.0).astype(bf16)
         for j in range(4)], axis=1)  # [128, 4, 512]
    cm = np.ascontiguousarray(
        np.broadcast_to(cm[:, :, None, :], (P, 4, HPC, 512)))

    Wq = (np.asarray(Wq, np.float32) * scale).astype(bf16)
    bq = np.asarray(bq, np.float32) * scale
    Wk = np.asarray(Wk, np.float32).astype(bf16)
    bk = np.asarray(bk, np.float32)
    Wv = np.asarray(Wv, np.float32).astype(bf16)
    bv = np.asarray(bv, np.float32)
    Wp = np.asarray(Wp, np.float32).astype(bf16)

    def wrearr(w, cs):
        # [1024, 128] core slice -> [p, o, m] = W[o*128+p, m]
        return np.ascontiguousarray(
            w[:, cs].reshape(KD8, P, P).transpose(1, 0, 2))

    in_maps = []
    for c in range(NCORES):
        cs = slice(c * P, (c + 1) * P)
        in_maps.append({
            "xT": xTc,
            "wq": wrearr(Wq, cs),
            "wk": wrearr(Wk, cs),
            "wv": wrearr(Wv, cs),
            "bq": np.ascontiguousarray(bq[cs].reshape(P, 1)),
            "bk": np.ascontiguousarray(bk[cs].reshape(P, 1)),
            "bv": np.ascontiguousarray(bv[cs].reshape(P, 1)),
            "wp": np.ascontiguousarray(Wp[cs, :]),
            "maskb": mb,
            "cmask": cm,
        })
    return in_maps


def run(inputs, trace=False, tmpdir=None):
    """Compile (cached) + run on 8 cores. Returns (output, BassKernelResults)."""
    from concourse import bass_utils
    nc = _get_nc()
    in_maps = make_in_maps(**inputs)
    kwargs = {}
    if trace:
        kwargs = dict(trace=True, tmpdir=tmpdir)
    res = bass_utils.run_bass_kernel_spmd(
        nc, in_maps, core_ids=list(range(NCORES)), **kwargs)
    acc = np.zeros((BS, D), dtype=np.float32)
    for r in res.results:
        acc += r["yp"].astype(np.float32)
    out = acc + np.asarray(inputs["bp"], np.float32)[None, :]
    return out.reshape(B, S, D), res


def kernel(**inputs) -> np.ndarray:
    out, _ = run(inputs, trace=False)
    return out
